# revision 35
# baseline (speedup 1.0000x reference)
"""Trainium2 Bass kernel for the LocalizeModule retrieval problem.

Computation (reference):
    f  = relu(feat @ W1.T + b1) @ W2.T + b2        # [F, H]
    k  = keyword @ Wk.T + bk                       # [K, H]
    out = (cos_sim(k, f) + 1) * 0.49               # [K, F]

Sharding across 8 cores: frames (F) are sharded for the MLP and the
score GEMM, so each core emits the [K, F/8] score tile and the host
concatenates along F.  The keyword projection is split hybrid: the
first G/8 of the keywords are computed sharded (each core projects
G*64 keywords) and exchanged with a single AllGather (fp8 kT shards
with the f32 inverse norms bitcast into the tail of the payload); the
remaining (8-G)/8 are computed replicated on every core.  G trades
PE work against collective exposure: the cold-start AllGather costs
~30us entry + ~4us/MB on this fabric, and running collectives also
GPIO-throttles the PE to ~80%, so the gather must be fully hidden
behind the replicated keyword projection + frame MLP + the replicated
score tiles (which are ordered first).

On-chip layout is "transposed" (H on partitions) throughout so that all
matmuls contract over the partition dimension and biases are
per-partition.  Row norms are computed with a ones-vector matmul over
squared activations; the (1/norm) factors are applied in the score
epilogue (per-partition for keywords, replicated-broadcast for frames).
All GEMMs run in fp8-e4m3 with DoubleRow (2x PE throughput) and fp32
PSUM accumulation; the output is written fp16 and upcast on host.
"""

import numpy as np
import ml_dtypes

import concourse.bass as bass  # noqa: F401  (bass types used via tile/bacc)
import concourse.mybir as mybir
import concourse.tile as tile
from concourse import bacc
from concourse.bass_utils import run_bass_kernel_spmd

P = 128
H = 1024
F = 8192
K = 4096
NCORES = 8
FS = F // NCORES          # 1024 frames per core
HO = H // P               # 8 partition chunks of the hidden dim
NCH = 512                 # matmul moving/free chunk (one PSUM bank of fp32)
F_CHUNKS = FS // NCH      # 2
K_TILES = K // P          # 32
EPS = 1e-8
OUT_SCALE = 0.49

import os as _os

# G of 8 keyword shards are gathered (computed K/8-sharded), 8-G replicated.
G = int(_os.environ.get("KERNEL_G", "8"))
assert G in (2, 4, 6, 8)
R = NCORES - G
GK = G * (NCH // NCORES)  # gathered keywords computed per core (G*64)
NSUB_G = GK // P          # their norm columns (G/2)
AGW = HO * GK + 32        # AllGather payload bytes/partition (norms + pad)
GKW = G * NCH             # total gathered keywords
RKW = R * NCH             # total replicated keywords

BF16 = mybir.dt.bfloat16
FP8 = mybir.dt.float8e4
F32 = mybir.dt.float32
F16 = mybir.dt.float16
AF = mybir.ActivationFunctionType
ALU = mybir.AluOpType

# which GEMM stages run in fp8-e4m3 with DoubleRow (2x PE throughput)
_FP8_STAGES = frozenset(
    s for s in _os.environ.get("KERNEL_FP8", "mlp,klin,score").split(",") if s
)
MLP_FP8 = "mlp" in _FP8_STAGES
KLIN_FP8 = "klin" in _FP8_STAGES
SCORE_FP8 = "score" in _FP8_STAGES

_CACHE = {}

LAST_EXEC_NS = None
LAST_RESULTS = None

RG = [list(range(NCORES))]


def _emit(tc, io):
    nc = tc.nc
    (featT_d, kwo_d, kwr_d, w1t_d, w2t_d, wkt_d, b1_d, b2_d, bk_d,
     out_d) = io

    import contextlib

    MLP_DT = FP8 if MLP_FP8 else BF16
    KLIN_DT = FP8 if KLIN_FP8 else BF16
    SC_DT = FP8 if SCORE_FP8 else BF16

    def mm_accum(ps, lhs_t, lhs_sl, rhs_t, rhs_sl, fp8):
        """Accumulate over the HO axis; fp8 stages use DoubleRow pairs."""
        step = 2 if fp8 else 1
        n = HO // step
        pm = mybir.MatmulPerfMode.DoubleRow if fp8 else None
        for i in range(n):
            ho = i * step
            if fp8:
                lhs = lhs_t[:, ho:ho + 2, lhs_sl]
                rhs = rhs_t[:, ho:ho + 2, rhs_sl]
            else:
                lhs = lhs_t[:, ho, lhs_sl]
                rhs = rhs_t[:, ho, rhs_sl]
            nc.tensor.matmul(
                ps, lhs, rhs, start=(i == 0), stop=(i == n - 1), perf_mode=pm
            )

    with contextlib.ExitStack() as ctx:
        const = ctx.enter_context(tc.tile_pool(name="const", bufs=1))
        psum = ctx.enter_context(tc.tile_pool(name="psum", bufs=1, space="PSUM"))
        dram = ctx.enter_context(tc.tile_pool(name="dram", bufs=1, space="DRAM"))
        fn_dram = dram.tile([P, HO], F32)          # bounce for the rfn transpose
        # collective bounce buffers (HBM->HBM AllGather; the gather
        # concatenates rank blocks along dim 0).  uint8: the payload mixes
        # fp8 keywords with bitcast f32 norms, and an fp8-typed collective
        # canonicalizes NaN byte patterns in flight, corrupting the norms.
        U8 = mybir.dt.uint8
        ag_in = dram.tile([P, AGW], U8)
        ag_out = dram.tile([NCORES, P, AGW], U8, addr_space="Shared")

        # ---- persistent SBUF tensors -------------------------------------
        w1t_s = const.tile([P, HO, H], MLP_DT)
        w2t_s = const.tile([P, HO, H], MLP_DT)
        wkt_s = const.tile([P, HO, H], KLIN_DT)
        featT_s = const.tile([P, HO, FS], MLP_DT)
        kw_own = const.tile([P, HO, GK], KLIN_DT)  # raw keywords, own G-shard
        kT_own = const.tile([P, HO * GK], KLIN_DT)  # projected own shard, flat
        if R:
            kw_rep = const.tile([P, HO, RKW], KLIN_DT)  # raw keywords, replicated
        b1_s = const.tile([P, HO], F32)
        b2_s = const.tile([P, HO], F32)
        bk_s = const.tile([P, HO], F32)
        ones_s = const.tile([P, 1], BF16)
        hT_s = const.tile([P, HO, FS], MLP_DT)     # relu(W1 @ featT + b1)
        fT_s = const.tile([P, HO, FS], SC_DT)      # projected frames, transposed
        # gathered keywords land block-major: each rank's [HO, GK] shard is
        # contiguous per partition, so the landing DMA moves 3KB bursts
        # instead of HO strided 384B ones
        kT_g = const.tile([P, NCORES, HO, GK], SC_DT)
        if R:
            kT_r = const.tile([P, HO, RKW], SC_DT)  # replicated keywords
        fnp_raw = const.tile([P, HO], F32)         # ||f_j||, partition-major per f-tile
        rfnp = const.tile([P, HO], F32)            # 0.49 / ||f_j||, partition-major
        rfn_row = const.tile([1, FS], F32)
        rfn_b = const.tile([P, FS], F32)           # 0.49 / ||f_j||, replicated on partitions
        knp_own = const.tile([P, NSUB_G], F32)
        rkn_own = const.tile([P, 8], F32)          # 32B: fills the payload tail
        rkn_g = const.tile([P, NCORES, NSUB_G], F32)
        if R:
            knp_rep = const.tile([P, R * (NCH // P)], F32)
            rkn_r = const.tile([P, R * (NCH // P)], F32)

        bias049_s = const.tile([P, 1], F32)
        nc.vector.memset(bias049_s[:], OUT_SCALE)
        nc.vector.memset(ones_s[:], 1.0)
        nc.vector.memset(rkn_own[:], 0.0)

        # ---- head DMAs ---------------------------------------------------
        # Two HWDGE queues (Sync + Scalar).  The keyword projection is first
        # on the PE and its first PSUM tile already contracts over all of
        # HO, so wkt + the own keyword shard are split across both queues up
        # front.  Sync then stops at the ag_in bounce (it waits for kproj to
        # finish, ~20us) so the collective doorbell rings as early as
        # possible; everything else rides the Scalar queue in consumption
        # order.
        nc.sync.dma_start(wkt_s[:, 0:4], wkt_d[:, 0:4])
        nc.sync.dma_start(kw_own[:, 0:4], kwo_d[:, 0:4])
        nc.sync.dma_start(bk_s[:], bk_d[:])
        nc.scalar.dma_start(wkt_s[:, 4:8], wkt_d[:, 4:8])
        nc.scalar.dma_start(kw_own[:, 4:8], kwo_d[:, 4:8])
        nc.scalar.dma_start(b1_s[:], b1_d[:])
        if R:
            # block 0 is needed the moment kproj-own drains (~21us); split
            # its halves across both queues ahead of the w1t/featT stream
            nc.sync.dma_start(kw_rep[:, 0:4, 0:NCH], kwr_d[:, 0:4, 0:NCH])
            nc.scalar.dma_start(kw_rep[:, 4:8, 0:NCH], kwr_d[:, 4:8, 0:NCH])
            if RKW > NCH:
                nc.scalar.dma_start(kw_rep[:, 0:4, NCH:RKW],
                                    kwr_d[:, 0:4, NCH:RKW])
                nc.scalar.dma_start(kw_rep[:, 4:8, NCH:RKW],
                                    kwr_d[:, 4:8, NCH:RKW])
        nc.scalar.dma_start(w1t_s[:, 0:4], w1t_d[:, 0:4])
        nc.scalar.dma_start(featT_s[:, 0:4, 0:NCH], featT_d[:, 0:4, 0:NCH])
        nc.scalar.dma_start(w1t_s[:, 4:8], w1t_d[:, 4:8])
        nc.scalar.dma_start(featT_s[:, 4:8, 0:NCH], featT_d[:, 4:8, 0:NCH])
        # featT tails / b2 / w2t are deliberately NOT emitted here: the tile
        # framework assigns DMA completion semaphores round-robin and
        # computes wait thresholds in emission order, so late head loads
        # emitted before the collective leak into the collective doorbell's
        # wait conditions (observed pinning the doorbell to ~35us while
        # w2t finished).  They are emitted right after the AllGather below.

        with tc.tile_pool(name="work", bufs=1) as work:

            def new_sq(w, name="sq"):
                return work.tile([P, HO, w], BF16, tag="sq", bufs=3, name=name)

            def emit_tree_sum(sq, w):
                """Pairwise-tree DVE sum of sq over the HO axis (depth 3)."""
                tmps = []
                for i in range(HO // 2):
                    tmp = work.tile([P, w], BF16, tag="tsum", bufs=4, name="tsum")
                    nc.vector.tensor_tensor(
                        tmp[:], sq[:, 2 * i, :], sq[:, 2 * i + 1, :], ALU.add
                    )
                    tmps.append(tmp)
                nc.vector.tensor_tensor(tmps[0][:], tmps[0][:], tmps[1][:], ALU.add)
                nc.vector.tensor_tensor(tmps[2][:], tmps[2][:], tmps[3][:], ALU.add)
                ssum = work.tile([P, w], BF16, tag="sqs", bufs=3, name="ssum")
                nc.vector.tensor_tensor(ssum[:], tmps[0][:], tmps[2][:], ALU.add)
                return ssum

            def emit_norm_mms(ssum, dst, cols, nsub):
                # row norms, partition-major: squares as lhsT -> psum
                # [P, nsub]; the column-disjoint matmuls share one PSUM tile
                # so they don't serialize on a consumer between them
                n_ps = psum.tile([P, nsub], F32, tag="cn", bufs=1, name="n_ps")
                for sub in range(nsub):
                    nc.tensor.matmul(
                        n_ps[:, sub:sub + 1],
                        ssum[:, sub * P:(sub + 1) * P],
                        ones_s[:],
                        start=True,
                        stop=True,
                    )
                nc.scalar.sqrt(dst[:, cols], n_ps[:])

            # ---- keyword projection: own gathered shard ----------------------
            # each mo slab is shipped to the bounce buffer as soon as it
            # drains, so the collective doorbell only waits on the last slab
            # + the norms instead of a full 512KB transfer
            sqk = new_sq(GK, "sqk")
            for mo in range(HO):
                kk_ps = psum.tile([P, GK], F32, tag="mm", bufs=6, name="kk_ps")
                mm_accum(kk_ps[:], wkt_s, slice(mo * P, (mo + 1) * P),
                         kw_own, slice(0, GK), KLIN_FP8)
                k_sl = kT_own[:, mo * GK:(mo + 1) * GK]
                nc.scalar.activation(
                    k_sl, kk_ps[:], AF.Identity,
                    bias=bk_s[:, mo:mo + 1], scale=1.0,
                )
                if mo in (1, 4, 7):
                    nc.scalar.square(sqk[:, mo, :], k_sl)
                else:
                    nc.vector.tensor_tensor(sqk[:, mo, :], k_sl, k_sl, ALU.mult)
                if mo in (3, 7):
                    # exactly TWO payload slabs (+ the norms DMA below): the
                    # DMA sem pool holds ~10 sems assigned round-robin in
                    # emission order, and the head emits 10 DMAs, so a
                    # 3-DMA payload inherits sems from the FIRST three head
                    # loads (complete ~15us) instead of the last ones
                    # (~28us) — which otherwise gate the doorbell at ~33us.
                    nc.sync.dma_start(
                        ag_in[:, (mo - 3) * GK:(mo + 1) * GK].bitcast(KLIN_DT),
                        kT_own[:, (mo - 3) * GK:(mo + 1) * GK],
                    )
            ssum_own = emit_tree_sum(sqk, GK)
            emit_norm_mms(ssum_own, knp_own, slice(0, NSUB_G), NSUB_G)
            nc.vector.tensor_scalar_max(knp_own[:], knp_own[:], EPS)
            nc.vector.reciprocal(rkn_own[:, 0:NSUB_G], knp_own[:])

            # ship the norms (bitcast into the payload tail) and start the
            # gather; TOPSP+SDMA move the bytes while the PE runs the
            # replicated projection + frame MLP + replicated score tiles
            nc.sync.dma_start(
                ag_in[:, HO * GK:AGW].bitcast(F32),
                rkn_own[:],
            )
            nc.gpsimd.collective_compute(
                "AllGather", ALU.bypass, replica_groups=RG,
                ins=[ag_in[:]], outs=[ag_out[:]],
            )

            # late head loads, emitted after the collective so their DMA
            # semaphores cannot gate the doorbell; first needed at MLP1
            # chunk 1 (~30us) and MLP2 (~37us).  w2t rides the Sync queue,
            # which is idle once the payload has shipped.
            nc.scalar.dma_start(featT_s[:, 0:4, NCH:FS], featT_d[:, 0:4, NCH:FS])
            nc.scalar.dma_start(featT_s[:, 4:8, NCH:FS], featT_d[:, 4:8, NCH:FS])
            nc.scalar.dma_start(b2_s[:], b2_d[:])
            nc.sync.dma_start(w2t_s[:, 0:4], w2t_d[:, 0:4])
            nc.sync.dma_start(w2t_s[:, 4:8], w2t_d[:, 4:8])

            # ---- keyword projection: replicated tail -------------------------
            rep_ssums = []
            for b in range(R):
                sqr = new_sq(NCH, "sqr")
                for mo in range(HO):
                    kr_ps = psum.tile([P, NCH], F32, tag="mm", bufs=6, name="kr_ps")
                    mm_accum(kr_ps[:], wkt_s, slice(mo * P, (mo + 1) * P),
                             kw_rep, slice(b * NCH, (b + 1) * NCH), KLIN_FP8)
                    k_sl = kT_r[:, mo, b * NCH:(b + 1) * NCH]
                    nc.scalar.activation(
                        k_sl, kr_ps[:], AF.Identity,
                        bias=bk_s[:, mo:mo + 1], scale=1.0,
                    )
                    if mo in (1, 4, 7):
                        nc.scalar.square(sqr[:, mo, :], k_sl)
                    else:
                        nc.vector.tensor_tensor(sqr[:, mo, :], k_sl, k_sl, ALU.mult)
                rep_ssums.append(emit_tree_sum(sqr, NCH))

            nsub = NCH // P

            def emit_rep_norms(b):
                # keyword norms for replicated block b -> rkn_r columns
                cols = slice(b * nsub, (b + 1) * nsub)
                emit_norm_mms(rep_ssums[b], knp_rep, cols, nsub)
                nc.vector.tensor_scalar_max(knp_rep[:, cols], knp_rep[:, cols], EPS)
                nc.vector.reciprocal(rkn_r[:, cols], knp_rep[:, cols])

            # ---- MLP layer 1 -------------------------------------------------
            # replicated-block norms are emitted one phase late so the PE
            # never waits on the DVE square/tree pipeline
            for c in range(F_CHUNKS):
                for mo in range(HO):
                    h1_ps = psum.tile([P, NCH], F32, tag="mm", bufs=6, name="h1_ps")
                    mm_accum(h1_ps[:], w1t_s, slice(mo * P, (mo + 1) * P),
                             featT_s, slice(c * NCH, (c + 1) * NCH), MLP_FP8)
                    nc.scalar.activation(
                        hT_s[:, mo, c * NCH:(c + 1) * NCH],
                        h1_ps[:],
                        AF.Relu,
                        bias=b1_s[:, mo:mo + 1],
                        scale=1.0,
                    )
                for b in range(R):
                    if b % F_CHUNKS == c:
                        emit_rep_norms(b)

            # ---- MLP layer 2 + frame squares + norms -------------------------
            # PSUM consumer (bias add + fp8 store) on Scalar, squares on
            # Vector (fp8-rate) — keeps Vector from becoming the PSUM gate
            f_ssums = []
            for c in range(F_CHUNKS):
                sq = new_sq(NCH, "sqf")
                for mo in range(HO):
                    f2_ps = psum.tile([P, NCH], F32, tag="mm", bufs=6, name="f2_ps")
                    mm_accum(f2_ps[:], w2t_s, slice(mo * P, (mo + 1) * P),
                             hT_s, slice(c * NCH, (c + 1) * NCH), MLP_FP8)
                    f_sl = fT_s[:, mo, c * NCH:(c + 1) * NCH]
                    nc.scalar.activation(
                        f_sl, f2_ps[:], AF.Identity,
                        bias=b2_s[:, mo:mo + 1], scale=1.0,
                    )
                    # V carries squares + tree; shift two squares to Scalar
                    # to balance
                    if mo in (2, 5):
                        nc.scalar.square(sq[:, mo, :], f_sl)
                    else:
                        nc.vector.tensor_tensor(sq[:, mo, :], f_sl, f_sl, ALU.mult)
                f_ssums.append(emit_tree_sum(sq, NCH))
                if c == 1:
                    # frame norms for chunk 0 while chunk 1's tree finishes
                    # 0.49/max(sqrt(n),eps) == 1/max(sqrt(n/0.49^2), eps/0.49)
                    nf_ps = psum.tile([P, nsub], F32, tag="cn", bufs=1, name="nf_ps")
                    for sub in range(nsub):
                        nc.tensor.matmul(
                            nf_ps[:, sub:sub + 1],
                            f_ssums[0][:, sub * P:(sub + 1) * P],
                            ones_s[:], start=True, stop=True,
                        )
                    nc.scalar.activation(
                        fnp_raw[:, 0:nsub], nf_ps[:], AF.Sqrt,
                        bias=0.0, scale=1.0 / (OUT_SCALE * OUT_SCALE),
                    )

            nf_ps = psum.tile([P, nsub], F32, tag="cn", bufs=1, name="nf_ps")
            for sub in range(nsub):
                nc.tensor.matmul(
                    nf_ps[:, sub:sub + 1],
                    f_ssums[1][:, sub * P:(sub + 1) * P],
                    ones_s[:], start=True, stop=True,
                )
            nc.scalar.activation(
                fnp_raw[:, nsub:2 * nsub], nf_ps[:], AF.Sqrt,
                bias=0.0, scale=1.0 / (OUT_SCALE * OUT_SCALE),
            )
            # [P, 8] partition-parallel max/recip (a free-major [1, F]
            # chain costs ~8us of single-lane DVE); the partition->free
            # transpose bounces through DRAM, where APs are unconstrained.
            # Emitted on Sync BEFORE the gather-landing DMAs: the replicated
            # score epilogues need rfn_b well before the collective is done.
            nc.vector.tensor_scalar_max(fnp_raw[:], fnp_raw[:], EPS / OUT_SCALE)
            nc.vector.reciprocal(rfnp[:], fnp_raw[:])
            nc.sync.dma_start(fn_dram[:], rfnp[:])
            nc.sync.dma_start(rfn_row[0:1, :], fn_dram[:].rearrange("p s -> s p"))
            nc.gpsimd.partition_broadcast(rfn_b[:], rfn_row[:])

            # ---- land the gathered keywords + norms --------------------------
            # block-major destination: each rank's shard is one contiguous
            # 3KB-per-partition burst.  With no replicated tail (G=8) the
            # Scalar engine has nothing to do between MLP2 and the first
            # post-gather epilogue, so the landing is split across both HWDGE
            # queues (even blocks on Sync, odd on Scalar) to halve its span.
            for r in range(NCORES):
                eng = nc.scalar if (R == 0 and r % 2 == 1) else nc.sync
                eng.dma_start(
                    kT_g[:, r],
                    ag_out[r, :, 0:HO * GK].bitcast(KLIN_DT)
                    .rearrange("p (h k) -> p h k", h=HO),
                )
                eng.dma_start(
                    rkn_g[:, r],
                    ag_out[r, :, HO * GK:HO * GK + 4 * NSUB_G].bitcast(F32),
                )

            # ---- score GEMM + epilogue -----------------------------------
            # replicated keyword tiles first: they depend only on local
            # state, so the PE keeps streaming while the gather lands
            step = 2 if SCORE_FP8 else 1
            n_acc = HO // step
            pm = mybir.MatmulPerfMode.DoubleRow if SCORE_FP8 else None
            tile_order = list(range(G * 4, K_TILES)) + list(range(G * 4))
            for t in tile_order:
                s_pss = [
                    psum.tile([P, NCH], F32, tag="mm", bufs=6, name="s_ps")
                    for _ in range(F_CHUNKS)
                ]
                # chunk-outer: bank c=0 finishes accumulating 4 MMs before
                # bank c=1, so its epilogue starts ~0.9us earlier and the
                # V/S consumer load spreads instead of bursting per tile
                if t < G * 4:
                    blk, sub = t // NSUB_G, t % NSUB_G
                    kt_of = lambda ho: kT_g[:, blk, ho:ho + 2,
                                            sub * P:(sub + 1) * P]
                    kt_of1 = lambda ho: kT_g[:, blk, ho, sub * P:(sub + 1) * P]
                    rkn_ap = rkn_g[:, blk, sub:sub + 1]
                else:
                    tr = t - G * 4
                    kt_of = lambda ho: kT_r[:, ho:ho + 2, tr * P:(tr + 1) * P]
                    kt_of1 = lambda ho: kT_r[:, ho, tr * P:(tr + 1) * P]
                    rkn_ap = rkn_r[:, tr:tr + 1]
                for c in range(F_CHUNKS):
                    for i in range(n_acc):
                        ho = i * step
                        if SCORE_FP8:
                            lhs = kt_of(ho)
                            rhs = fT_s[:, ho:ho + 2, c * NCH:(c + 1) * NCH]
                        else:
                            lhs = kt_of1(ho)
                            rhs = fT_s[:, ho, c * NCH:(c + 1) * NCH]
                        nc.tensor.matmul(
                            s_pss[c][:], lhs, rhs,
                            start=(i == 0), stop=(i == n_acc - 1),
                            perf_mode=pm,
                        )
                for c in range(F_CHUNKS):
                    s_ps = s_pss[c]
                    stage = work.tile([P, NCH], BF16, tag="stage", bufs=6, name="stage")
                    out_t = work.tile([P, NCH], F16, tag="out_t", bufs=16, name="out_t")
                    nc.vector.tensor_tensor(
                        stage[:],
                        s_ps[:],
                        rfn_b[:, c * NCH:(c + 1) * NCH],
                        ALU.mult,
                    )
                    # Scalar runs at parity with the PE here; push every
                    # 4th tile's finisher onto Vector, which has slack
                    if t % 4 == 3:
                        nc.vector.tensor_scalar(
                            out_t[:], stage[:],
                            rkn_ap, OUT_SCALE,
                            ALU.mult, ALU.add,
                        )
                    else:
                        nc.scalar.activation(
                            out_t[:],
                            stage[:],
                            AF.Identity,
                            bias=bias049_s[:, 0:1],
                            scale=rkn_ap,
                        )
                    nc.sync.dma_start(
                        out_d[t * P:(t + 1) * P, c * NCH:(c + 1) * NCH],
                        out_t[:],
                    )


def build():
    """Build + compile the (core-agnostic) Bass program once."""
    key = ("nc", MLP_FP8, KLIN_FP8, SCORE_FP8, G)
    if key in _CACHE:
        return _CACHE[key]
    MLP_DT = FP8 if MLP_FP8 else BF16
    KLIN_DT = FP8 if KLIN_FP8 else BF16
    nc = bacc.Bacc(
        "TRN2",
        target_bir_lowering=False,
        debug=False,
        enable_asserts=False,
        num_devices=NCORES,
    )
    featT_d = nc.dram_tensor("featT", [P, HO, FS], MLP_DT, kind="ExternalInput").ap()
    kwo_d = nc.dram_tensor("kwo", [P, HO, GK], KLIN_DT, kind="ExternalInput").ap()
    if R:
        kwr_d = nc.dram_tensor("kwr", [P, HO, RKW], KLIN_DT,
                               kind="ExternalInput").ap()
    else:
        kwr_d = None
    w1t_d = nc.dram_tensor("w1t", [P, HO, H], MLP_DT, kind="ExternalInput").ap()
    w2t_d = nc.dram_tensor("w2t", [P, HO, H], MLP_DT, kind="ExternalInput").ap()
    wkt_d = nc.dram_tensor("wkt", [P, HO, H], KLIN_DT, kind="ExternalInput").ap()
    b1_d = nc.dram_tensor("b1t", [P, HO], F32, kind="ExternalInput").ap()
    b2_d = nc.dram_tensor("b2t", [P, HO], F32, kind="ExternalInput").ap()
    bk_d = nc.dram_tensor("bkt", [P, HO], F32, kind="ExternalInput").ap()
    out_d = nc.dram_tensor("out", [K, FS], F16, kind="ExternalOutput").ap()

    io = (featT_d, kwo_d, kwr_d, w1t_d, w2t_d, wkt_d, b1_d, b2_d, bk_d, out_d)
    with tile.TileContext(nc) as tc:
        _emit(tc, io)
    nc.compile()
    _CACHE[key] = nc
    return nc


def _part_tile(a):
    """[D0, rest...] with D0 = o*P + p  ->  [P, D0//P, rest...]"""
    d0 = a.shape[0]
    return np.ascontiguousarray(
        a.reshape(d0 // P, P, *a.shape[1:]).swapaxes(0, 1)
    )


def make_in_maps(feat, keyword, W1, b1, W2, b2, Wk, bk):
    mlp_np = ml_dtypes.float8_e4m3 if MLP_FP8 else ml_dtypes.bfloat16
    klin_np = ml_dtypes.float8_e4m3 if KLIN_FP8 else ml_dtypes.bfloat16
    feat = np.asarray(feat, np.float32)
    keyword = np.asarray(keyword, np.float32)
    kwT = _part_tile(np.ascontiguousarray(keyword.T)).astype(klin_np)   # [P, HO, K]
    w1t = _part_tile(np.ascontiguousarray(np.asarray(W1, np.float32).T)).astype(mlp_np)
    w2t = _part_tile(np.ascontiguousarray(np.asarray(W2, np.float32).T)).astype(mlp_np)
    wkt = _part_tile(np.ascontiguousarray(np.asarray(Wk, np.float32).T)).astype(klin_np)
    b1t = _part_tile(np.asarray(b1, np.float32))                        # [P, HO]
    b2t = _part_tile(np.asarray(b2, np.float32))
    bkt = _part_tile(np.asarray(bk, np.float32))

    kwr = np.ascontiguousarray(kwT[:, :, GKW:]) if R else None
    in_maps = []
    for c in range(NCORES):
        featT_c = _part_tile(
            np.ascontiguousarray(feat[c * FS:(c + 1) * FS, :].T)
        ).astype(mlp_np)                                                # [P, HO, FS]
        m = {
            "featT": featT_c,
            "kwo": np.ascontiguousarray(kwT[:, :, c * GK:(c + 1) * GK]),
            "w1t": w1t,
            "w2t": w2t,
            "wkt": wkt,
            "b1t": b1t,
            "b2t": b2t,
            "bkt": bkt,
        }
        if R:
            m["kwr"] = kwr
        in_maps.append(m)
    return in_maps


def kernel(feat, keyword, W1, b1, W2, b2, Wk, bk, _trace=False):
    global LAST_EXEC_NS, LAST_RESULTS
    nc = build()
    in_maps = make_in_maps(feat, keyword, W1, b1, W2, b2, Wk, bk)
    res = run_bass_kernel_spmd(
        nc,
        in_maps,
        core_ids=list(range(NCORES)),
        trace=_trace,
    )
    LAST_EXEC_NS = res.exec_time_ns
    LAST_RESULTS = res
    out = np.concatenate([res.results[c]["out"] for c in range(NCORES)], axis=1)
    return np.ascontiguousarray(out.astype(np.float32))
